# revision 1
# baseline (speedup 1.0000x reference)
"""Trainium2 Bass kernel for nn_BatchNeuralKB (batched Gaussian-kernel KNN max).

reference math:
    q = concat(rel, arg1, arg2)                 # [B, 384]
    f = concat(fact_rel, fact_arg1, fact_arg2)  # [F, 384]
    d2[b,i] = max(||q_b||^2 - 2 q_b.f_i + ||f_i||^2, 0)
    out[b]  = max_i exp(-d2[b,i] / 2)

Because exp is monotone decreasing, max_i exp(-d2/2) == exp(-max(min_i d2,0)/2),
so we only need a min-reduction over facts of (f_sq[i] - 2 q.f) and never
materialize exp over the [B, F] score matrix.

Distribution: fact table sharded across 8 NeuronCores along F (8192 facts
each), queries replicated.  Each core returns
    part[b] = exp(-max(min_{i in shard} d2[b,i], 0) / 2)
and the host takes the elementwise max over the 8 partials.

Per-core compute, all matmuls in fp8-e4m3 DoubleRow mode (0.5 cyc/row):
  mm1: pairs (rel, arg1):   lhsT = [-2 rel^T | -2 arg1^T],  rhs = [fr^T | fa1^T]
  mm2: pairs (arg2, f_sq):  lhsT = [-2 arg2^T | ones],      rhs = [fa2^T | fsqe]
where fsqe[e, i] = fr[i,e]^2 + fa1[i,e]^2 + fa2[i,e]^2 elementwise, so the
ones-block contracts it to  sum_e fsqe = ||f_i||^2  broadcast over all output
partitions.  PSUM thus accumulates  f_sq - 2 q.f  with no extra engine passes.

Reduction (DVE tensor_reduce has no 2x mode, tensor_tensor has 2x for bf16;
DVE may read at most one PSUM operand per op):
  L0: per pair of [128,1024] PSUM tiles, ACT drains one (or both, on half
      the groups, so the DVE min runs in bf16 2x mode) and a DVE
      tensor_tensor(min) merges the pair to bf16 SBUF,
  L1/L2: bf16 pairwise tensor_tensor(min) tree (2x_1p mode),
  per tile: fold [1024]->[512] (2x) + tensor_reduce(min) -> mn_all column.
Batched epilogue (all 2048 outputs at once): d2 = relu(mn_all + qsq_all) on
DVE, then one ACT Exp(-x/2).  q_sq comes from ACT Square with accum_out (sum
along free) on bf16 queries.
"""

import os
import numpy as np

B = 2048          # batch
E = 128           # embedding per part
F = 65536         # total facts
NCORES = 8
FS = F // NCORES  # facts per core
CHUNK = 512       # fact columns per matmul (PSUM bank = 512 fp32)
NCH = FS // CHUNK # 16 chunks
BT = 128          # batch tile (partition dim)
NBT = B // BT     # 16 batch tiles

_cache = {}


def _build_nc(repeat=1, phi_num=1, phi_den=3, l0_bufs=6, l1_bufs=6, ps_bufs=2, qsq_dve=False, loop_order='tg'):
    import concourse.bacc as bacc
    import concourse.tile as tile
    import concourse.mybir as mybir
    from contextlib import ExitStack

    f32 = mybir.dt.float32
    bf16 = mybir.dt.bfloat16
    f8 = mybir.dt.float8e4
    AF = mybir.ActivationFunctionType
    ALU = mybir.AluOpType
    AX = mybir.AxisListType
    DR = mybir.MatmulPerfMode.DoubleRow

    nc = bacc.Bacc("TRN2", target_bir_lowering=False, debug=False,
                   num_devices=NCORES)

    ft1_d = nc.dram_tensor("ft1", [NCH, E, 2, CHUNK], f8, kind="ExternalInput")
    ft2_d = nc.dram_tensor("ft2", [NCH, E, 2, CHUNK], f8, kind="ExternalInput")
    qd1_d = nc.dram_tensor("qd1", [E, 2, B], f8, kind="ExternalInput")
    qd2_d = nc.dram_tensor("qd2", [E, 2, B], f8, kind="ExternalInput")
    qn_d = nc.dram_tensor("qn", [B, 3 * E], bf16, kind="ExternalInput")
    out_d = nc.dram_tensor("out", [BT, NBT], f32, kind="ExternalOutput")

    with tile.TileContext(nc) as tc, ExitStack() as ctx:
        qt_p = ctx.enter_context(tc.tile_pool(name="qt", bufs=1))
        ft_p = ctx.enter_context(tc.tile_pool(name="ft", bufs=1))
        qn_p = ctx.enter_context(tc.tile_pool(name="qn", bufs=3))
        qscr_p = ctx.enter_context(tc.tile_pool(name="qscr", bufs=2))
        small_p = ctx.enter_context(tc.tile_pool(name="small", bufs=1))
        l0_p = ctx.enter_context(tc.tile_pool(name="l0", bufs=l0_bufs))
        l1_p = ctx.enter_context(tc.tile_pool(name="l1", bufs=l1_bufs))
        mm_ps = ctx.enter_context(tc.tile_pool(name="mm_ps", bufs=ps_bufs, space="PSUM"))

        qd1 = qt_p.tile([E, 2, B], f8, name="qd1", tag="qd1")
        qd2 = qt_p.tile([E, 2, B], f8, name="qd2", tag="qd2")
        nc.sync.dma_start(qd1[:], qd1_d[:])
        nc.sync.dma_start(qd2[:], qd2_d[:])

        # fact chunks as separate tiles for precise DMA->matmul deps
        ft1c = []
        ft2c = []
        for c in range(NCH):
            t1 = ft_p.tile([E, 2, CHUNK], f8, name=f"ft1_{c}", tag=f"ft1_{c}")
            t2 = ft_p.tile([E, 2, CHUNK], f8, name=f"ft2_{c}", tag=f"ft2_{c}")
            nc.sync.dma_start(t1[:], ft1_d[c])
            nc.sync.dma_start(t2[:], ft2_d[c])
            ft1c.append(t1)
            ft2c.append(t2)

        # q_sq per batch tile: ACT Square + accum(sum along free),
        # accumulated into columns of one wide tile for a batched epilogue
        qsq_all = small_p.tile([BT, NBT], f32, name="qsq_all", tag="qsq_all")
        for t in range(NBT):
            qnt = qn_p.tile([BT, 3 * E], bf16, name="qnt", tag="qnt")
            nc.sync.dma_start(qnt[:], qn_d[t * BT:(t + 1) * BT, :])
            scr = qscr_p.tile([BT, 3 * E], f32, name="qscr", tag="qscr")
            if qsq_dve:
                nc.vector.tensor_tensor(out=scr[:], in0=qnt[:], in1=qnt[:],
                                        op=ALU.mult)
                nc.vector.tensor_reduce(qsq_all[:, t:t + 1], scr[:],
                                        axis=AX.X, op=ALU.add)
            else:
                nc.scalar.activation(scr[:], qnt[:], AF.Square,
                                     accum_out=qsq_all[:, t:t + 1])

        out_all = small_p.tile([BT, NBT], f32, name="out_all", tag="out_all")
        mn_all = small_p.tile([BT, NBT], f32, name="mn_all", tag="mn_all")

        W = 2 * CHUNK  # 1024-wide (2 PSUM banks) drain granularity

        def mm_group(ps, tb, c):
            # two DR matmuls per 512-wide half; 2 halves = chunks c, c+1
            for h, cc in ((0, c), (1, c + 1)):
                sl = ps[:, h * CHUNK:(h + 1) * CHUNK]
                nc.tensor.matmul(sl, qd1[:, :, tb], ft1c[cc][:],
                                 start=True, stop=False, perf_mode=DR)
                nc.tensor.matmul(sl, qd2[:, :, tb], ft2c[cc][:],
                                 start=False, stop=True, perf_mode=DR)

        def drain_group(t, g, tb, m=None):
            # two 1024-wide PSUM tiles (4 chunks) merge into one bf16 tile.
            # DVE may read only ONE input from PSUM, so ACT drains the
            # other; on "double-ACT" groups ACT drains both and the DVE min
            # runs in 2x_1p bf16 mode.  The mix balances ACT vs DVE load.
            psA = mm_ps.tile([BT, W], f32, name="psA", tag="psA")
            psB = mm_ps.tile([BT, W], f32, name="psB", tag="psB")
            mm_group(psA, tb, 4 * g)
            mm_group(psB, tb, 4 * g + 2)
            if m is None:
                m = l0_p.tile([BT, W], bf16, name="l0m", tag="l0m")
            if (2 * t + g) % phi_den < phi_num:  # double-ACT group
                cp0 = l0_p.tile([BT, W], bf16, name="l0a", tag="l0a")
                cp1 = l0_p.tile([BT, W], bf16, name="l0b", tag="l0b")
                nc.scalar.copy(cp0[:], psA[:])
                nc.scalar.copy(cp1[:], psB[:])
                nc.vector.tensor_tensor(out=m[:], in0=cp0[:], in1=cp1[:],
                                        op=ALU.min)
            else:  # single-ACT group: DVE min vs PSUM at 1x
                cp1 = l0_p.tile([BT, W], bf16, name="l0b", tag="l0b")
                nc.scalar.copy(cp1[:], psB[:])
                nc.vector.tensor_tensor(out=m[:], in0=psA[:],
                                        in1=cp1[:], op=ALU.min)
            return m

        def finish_tile(t, merged):
            # fold [1024] -> [512] in 2x mode, then a small reduce into this
            # tile's column of mn_all; the +q_sq/clamp/exp epilogue runs once
            # over all 16 columns (saves ~30 tiny ACT ops)
            half = l1_p.tile([BT, CHUNK], bf16, name="half", tag="half")
            nc.vector.tensor_tensor(out=half[:], in0=merged[:, :CHUNK],
                                    in1=merged[:, CHUNK:], op=ALU.min)
            nc.vector.tensor_reduce(mn_all[:, t:t + 1], half[:], axis=AX.X,
                                    op=ALU.min)

        NG = NCH // 4
        for _rep in range(repeat):
          if loop_order == 'tg':
            for t in range(NBT):
                tb = slice(t * BT, (t + 1) * BT)
                l0 = [drain_group(t, g, tb) for g in range(NG)]
                # tensor_scalar min-accumulate runs at 4x on bf16 SBUF and
                # reduces each merged tile straight to a per-partition min,
                # replacing the whole pairwise tree + final reduce
                qm = small_p.tile([BT, NG], f32, name=f"qm{t}", tag=f"qm{t}")
                for g, m in enumerate(l0):
                    nc.vector.tensor_scalar(
                        out=m[:], in0=m[:], scalar1=3.0e38, scalar2=None,
                        op0=ALU.min, op1=ALU.min,
                        accum_out=qm[:, g:g + 1])
                nc.vector.tensor_reduce(mn_all[:, t:t + 1], qm[:], axis=AX.X,
                                        op=ALU.min)
          else:  # 'gt': group-outer, running min chained per batch tile
            runs = [None] * NBT
            for g in range(NG):
                for t in range(NBT):
                    tb = slice(t * BT, (t + 1) * BT)
                    if runs[t] is None:
                        runs[t] = drain_group(t, g, tb, m=l1_p.tile(
                            [BT, W], bf16, name=f"run{t}", tag=f"run{t}",
                            bufs=2))
                    else:
                        m = drain_group(t, g, tb)
                        nxt = l1_p.tile([BT, W], bf16, name=f"run{t}",
                                        tag=f"run{t}", bufs=2)
                        nc.vector.tensor_tensor(out=nxt[:], in0=runs[t][:],
                                                in1=m[:], op=ALU.min)
                        runs[t] = nxt
            for t in range(NBT):
                finish_tile(t, runs[t])

          # batched epilogue: +q_sq, clamp, exp -- all 2048 outputs in
          # three wide ops
          d2_all = small_p.tile([BT, NBT], f32, name="d2_all", tag="d2_all")
          nc.vector.tensor_tensor(out=d2_all[:], in0=mn_all[:],
                                  in1=qsq_all[:], op=ALU.add)
          nc.vector.tensor_scalar_max(d2_all[:], d2_all[:], 0.0)
          nc.scalar.activation(out_all[:], d2_all[:], AF.Exp, scale=-0.5)

        nc.sync.dma_start(out_d[:], out_all[:])

    nc.compile()
    return nc


def _get_nc(repeat=1):
    key = f"nc{repeat}"
    if key not in _cache:
        _cache[key] = _build_nc(repeat)
    return _cache[key]


def make_in_maps(rel, arg1, arg2, fact_rel, fact_arg1, fact_arg2):
    import ml_dtypes
    f8 = ml_dtypes.float8_e4m3

    q = [np.asarray(x, dtype=np.float32) for x in (rel, arg1, arg2)]
    f = [np.asarray(x, dtype=np.float32) for x in (fact_rel, fact_arg1, fact_arg2)]

    def stack2(a, b):  # [E, X] x2 -> [E, 2, X]
        return np.ascontiguousarray(np.stack([a, b], axis=1))

    qd1 = stack2((-2.0 * q[0]).T.astype(f8), (-2.0 * q[1]).T.astype(f8))
    qd2 = stack2((-2.0 * q[2]).T.astype(f8), np.ones((E, B), dtype=f8))
    import ml_dtypes as _md
    qn = np.ascontiguousarray(np.concatenate(q, axis=1).astype(_md.bfloat16))

    def chunked(a):  # [E, 2, FS] -> [NCH, E, 2, CHUNK] contiguous per chunk
        return np.ascontiguousarray(
            a.reshape(E, 2, NCH, CHUNK).transpose(2, 0, 1, 3))

    in_maps = []
    for c in range(NCORES):
        sh = [np.ascontiguousarray(x[c * FS:(c + 1) * FS].T) for x in f]
        fsqe = (sh[0] * sh[0] + sh[1] * sh[1] + sh[2] * sh[2]).astype(f8)
        in_maps.append({
            "ft1": chunked(stack2(sh[0].astype(f8), sh[1].astype(f8))),
            "ft2": chunked(stack2(sh[2].astype(f8), fsqe)),
            "qd1": qd1, "qd2": qd2, "qn": qn,
        })
    return in_maps


def kernel(rel, arg1, arg2, fact_rel, fact_arg1, fact_arg2):
    from concourse.bass_utils import run_bass_kernel_spmd

    in_maps = make_in_maps(rel, arg1, arg2, fact_rel, fact_arg1, fact_arg2)
    nc = _get_nc()
    trace = bool(int(os.environ.get("KB_TRACE", "0")))
    try:
        res = run_bass_kernel_spmd(nc, in_maps, core_ids=list(range(NCORES)),
                                   trace=trace)
    except (ImportError, ModuleNotFoundError):
        # NTFF profile hook unavailable in this environment; run untraced.
        res = run_bass_kernel_spmd(nc, in_maps, core_ids=list(range(NCORES)),
                                   trace=False)
    _cache["last_result"] = res
    outs = [r["out"].T.reshape(B) for r in res.results]
    return np.maximum.reduce(outs).astype(np.float32)

